# revision 33
# baseline (speedup 1.0000x reference)
"""Distributed multi-head causal attention for TRN2, 8 NeuronCores.

Strategy (tensor-parallel over heads + AllToAll re-shard for the output
projection):
  - Each core owns 2 of the 16 heads. It computes Q,K,V projections for its
    heads over the full sequence (both batches), applies RoPE, and computes
    causal softmax(QK^T/sqrt(hd)) @ V for its heads.
  - Everything on-chip is laid out TRANSPOSED: qT/kT are [hd, B*S], scores are
    [k, q], attention output is [hd, q].  This avoids all transposes:
      scoresT = kT_block.T @ qT        (lhsT=kT block, rhs=qT)
      aoT     = v_block.T  @ pT        (lhsT=v natural [k,hd], rhs=pT [k,q])
    Softmax denominator (sum over k = partition axis) comes from a ones-matmul
    (lhsT=ones [128,128]) that also broadcasts the sum across partitions.
    exp() is computed WITHOUT max subtraction (max |score| ~ 6, safe in f32).
  - Four AllToAlls (one per (batch, head)) swap head-sharding for
    sequence-sharding; each fires as soon as that (batch, head)'s attention
    output is stored, overlapping the collective with the remaining attention
    compute.  Each core ends with all 16 heads for its 256-position slice of
    each batch, then computes its slice of the wo projection:
    outT = woT_chunk.T @ attnT (output transposed; host transposes back).
    The wo contraction runs in two passes over head parity (one PSUM bank
    per open accumulation group — start=True resets the whole bank), so the
    even-head half can run before the last AllToAll lands.
  - Compute dtype: bf16 matmul operands, f32 PSUM accumulation, f32 softmax,
    bf16 output (host upcasts; well within tolerance).
  - Startup: sync+scalar DMA queues carry only the x stream, gpsimd carries
    weights/grids/mask in demand order, so the first QK accumulation starts
    as early as the DMA-latency floor allows.

Host-side prep casts inputs to bf16 and pre-transposes x/wo; host-side
assembly transposes/concats per-core outputs.  No host arithmetic.
"""
import math

import ml_dtypes
import numpy as np

import concourse.bass as bass
import concourse.mybir as mybir
from concourse import bacc
from concourse.tile import TileContext

F32 = mybir.dt.float32
BF16 = mybir.dt.bfloat16

N_CORES = 8
CORE_IDS = list(range(N_CORES))
B = 2
S = 2048
D = 2048
H = 16
HD = 128  # head dim
HPC = H // N_CORES  # heads per core = 2
BS = B * S  # 4096
NB = S // 512  # 4 q-free-blocks per batch
NK = S // 128  # 16 k-blocks per batch
SCHUNK = S // N_CORES  # 256 positions per core per batch
INV_SQRT_HD = 1.0 / math.sqrt(HD)

# stream_shuffle mask: swap adjacent partitions within each 32-group
PAIR_SWAP = [i ^ 1 for i in range(32)]


def build():
    nc = bacc.Bacc(None, num_devices=N_CORES)

    # x pre-transposed/chunked: [half, j2, p, i, n] fully contiguous per
    # (half, j2) so startup DMAs are large sequential reads.
    xt = nc.declare_dram_parameter("xt", [2 * B, 2, 128, 16, 512], BF16, isOutput=False)
    # per-head weight chunks: [h, p, i, hd] contiguous per head.
    wqt = nc.declare_dram_parameter("wqt", [HPC, 128, 16, HD], BF16, isOutput=False)
    wkt = nc.declare_dram_parameter("wkt", [HPC, 128, 16, HD], BF16, isOutput=False)
    wvt = nc.declare_dram_parameter("wvt", [HPC, 128, 16, HD], BF16, isOutput=False)
    wot = nc.declare_dram_parameter("wot", [128, 16, D], BF16, isOutput=False)
    cgrid = nc.declare_dram_parameter("cgrid", [HD, S], F32, isOutput=False)
    sgrid = nc.declare_dram_parameter("sgrid", [HD, S], F32, isOutput=False)
    masks = nc.declare_dram_parameter("masks", [HD, 2, 1024], BF16, isOutput=False)
    out_ext = nc.declare_dram_parameter("out", [D, B * SCHUNK], BF16, isOutput=True)

    bnc_in = [
        [nc.dram_tensor(f"bounce_in{b}_{h}", [N_CORES, HD, SCHUNK], BF16)
         for h in range(HPC)]
        for b in range(B)
    ]
    bnc_out = [
        [nc.dram_tensor(f"bounce_out{b}_{h}", [N_CORES, HD, SCHUNK], BF16)
         for h in range(HPC)]
        for b in range(B)
    ]

    bar_in = nc.dram_tensor("bar_in", [1], F32)
    bar_out = nc.dram_tensor("bar_out", [N_CORES], F32, addr_space="Shared")

    with TileContext(nc) as tc:
        with (
            tc.tile_pool(name="persist", bufs=1) as persist,
            tc.tile_pool(name="tmp", bufs=4) as tmp,
        ):
            # ---------------- persistent SBUF tensors ----------------
            mask_sb = persist.tile([128, 2, 1024], BF16, tag="mask")
            ones_sb = persist.tile([128, 128], BF16, tag="ones")
            nc.vector.memset(ones_sb, 1.0)

            # qT/kT per head: [hd=128, BS] bf16 (post-RoPE).
            # v per head: [128, BS] bf16, chunk ik at cols [128*ik,128*(ik+1))
            # holding v rows (k) on partitions, hd on free.
            q_sb = [persist.tile([128, BS], BF16, tag=f"q{h}", name=f"q_sb{h}") for h in range(HPC)]
            k_sb = [persist.tile([128, BS], BF16, tag=f"k{h}", name=f"k_sb{h}") for h in range(HPC)]
            v_sb = [persist.tile([128, BS], BF16, tag=f"v{h}", name=f"v_sb{h}") for h in range(HPC)]

            # ---------------- phase 1: QKV projections + RoPE ----------------
            with (
                tc.tile_pool(name="p1w", bufs=1) as p1w,
                tc.tile_pool(name="xt_pool", bufs=2) as xt_pool,
                tc.tile_pool(name="p1psum", bufs=1, space="PSUM") as p1psum,
                tc.tile_pool(name="p1psumv", bufs=2, space="PSUM") as p1psumv,
            ):
                wq_sb = p1w.tile([128, 16, HPC * HD], BF16, tag="wq")
                wk_sb = p1w.tile([128, 16, HPC * HD], BF16, tag="wk")
                wv_sb = p1w.tile([128, 16, HPC * HD], BF16, tag="wv")
                cg_sb = p1w.tile([128, S], F32, tag="cg")
                sg_sb = p1w.tile([128, S], F32, tag="sg")

                # Startup DMA priority: the sync+scalar queues carry ONLY the
                # xt stream (so nothing backs it up); gpsimd carries weights +
                # grids + mask in exact demand order.  The first xt chunk is
                # split 4 ways i-ascending so the first QK accumulation can
                # begin as soon as the first i-chunks land.
                # Startup DMA priority: sync+scalar carry ONLY the xt stream;
                # gpsimd carries weights + grids + mask in demand order.
                xt00_sb = xt_pool.tile([128, 16, 512], BF16, tag="xt")
                nc.sync.dma_start(out=xt00_sb[:, 0:4, :], in_=xt[0, 0, :, 0:4, :])
                nc.scalar.dma_start(out=xt00_sb[:, 4:8, :], in_=xt[0, 0, :, 4:8, :])
                nc.sync.dma_start(out=xt00_sb[:, 8:12, :], in_=xt[0, 0, :, 8:12, :])
                nc.scalar.dma_start(out=xt00_sb[:, 12:16, :], in_=xt[0, 0, :, 12:16, :])
                nc.gpsimd.dma_start(out=wq_sb[:, :, 0:HD], in_=wqt[0])
                # dummy AllGather: absorbs cross-core NEFF-launch skew early,
                # so the later AllToAlls see aligned peers
                nc.gpsimd.collective_compute(
                    "AllGather",
                    mybir.AluOpType.bypass,
                    replica_groups=[CORE_IDS],
                    ins=[bar_in[:]],
                    outs=[bar_out[:]],
                )
                nc.gpsimd.dma_start(out=wq_sb[:, :, HD:2 * HD], in_=wqt[1])
                nc.gpsimd.dma_start(out=wk_sb[:, :, 0:HD], in_=wkt[0])
                nc.gpsimd.dma_start(out=wk_sb[:, :, HD:2 * HD], in_=wkt[1])
                # RoPE grids in demand order: first 512 cols feed the
                # first RoPE block; the rest can trickle in later.
                nc.gpsimd.dma_start(out=cg_sb[:, 0:512], in_=cgrid[:, 0:512])
                nc.gpsimd.dma_start(out=sg_sb[:, 0:512], in_=sgrid[:, 0:512])
                nc.gpsimd.dma_start(out=wv_sb[:, :, 0:HD], in_=wvt[0])
                nc.gpsimd.dma_start(out=wv_sb[:, :, HD:2 * HD], in_=wvt[1])
                nc.gpsimd.dma_start(out=cg_sb[:, 512:1024], in_=cgrid[:, 512:1024])
                nc.gpsimd.dma_start(out=sg_sb[:, 512:1024], in_=sgrid[:, 512:1024])
                nc.gpsimd.dma_start(out=cg_sb[:, 1024:2048], in_=cgrid[:, 1024:2048])
                nc.gpsimd.dma_start(out=sg_sb[:, 1024:2048], in_=sgrid[:, 1024:2048])
                nc.gpsimd.dma_start(out=mask_sb, in_=masks[:, :, :])

                # HAM clock-gate warmup: the PE runs at 1.2 GHz until it sees
                # ~3.4us of sustained activity, and re-cools after ~3.4us
                # idle.  The first real matmul waits ~14us for DMA; fill that
                # whole shadow with dummy matmuls on DMA-free tiles so real
                # work starts at full clock (cold: ~12 mm @427ns, then warm
                # @213ns up to ~22.5us).
                warm_src = p1w.tile([128, 512], BF16, tag="warmsrc")
                nc.vector.memset(warm_src, 1.0)
                warm = p1psum.tile([128, 512], F32, tag="warm")
                for i in range(68):
                    nc.tensor.matmul(
                        warm, ones_sb, warm_src,
                        start=(i == 0), stop=(i == 67),
                    )

                for half in range(2 * B):  # half-batches of 1024 positions
                    b, hf = divmod(half, 2)
                    for j2 in range(2):
                        coff = b * S + hf * 1024 + j2 * 512  # col off in [D, BS]
                        poff = hf * 1024 + j2 * 512  # position offset (grids)
                        if half == 0 and j2 == 0:
                            xt_sb = xt00_sb
                        else:
                            xt_sb = xt_pool.tile([128, 16, 512], BF16, tag="xt")
                            nc.sync.dma_start(
                                out=xt_sb[:, 0:8, :], in_=xt[half, j2, :, 0:8, :]
                            )
                            nc.scalar.dma_start(
                                out=xt_sb[:, 8:16, :], in_=xt[half, j2, :, 8:16, :]
                            )

                        # Q, K for both heads: psum [hd, 512] accum over d_in
                        ps = {}
                        for kind, w in (("q", wq_sb), ("k", wk_sb)):
                            for h in range(HPC):
                                p = p1psum.tile([128, 512], F32, tag=f"qk{kind}{h}")
                                ps[(kind, h)] = p
                                for i in range(16):
                                    nc.tensor.matmul(
                                        p,
                                        w[:, i, h * HD:(h + 1) * HD],
                                        xt_sb[:, i, :],
                                        start=(i == 0),
                                        stop=(i == 15),
                                    )
                        # RoPE: out = t*cos + pairswap(t)*sin_signed (DVE only)
                        gcol = slice(poff, poff + 512)
                        ocol = slice(coff, coff + 512)
                        for kind, dst in (("q", q_sb), ("k", k_sb)):
                            for h in range(HPC):
                                p = ps[(kind, h)]
                                m1 = tmp.tile([128, 512], F32, tag="rope_m1")
                                nc.vector.tensor_mul(m1, p, cg_sb[:, gcol])
                                sh = tmp.tile([128, 512], F32, tag="rope_sh")
                                nc.vector.stream_shuffle(sh, p, PAIR_SWAP)
                                nc.vector.tensor_mul(sh, sh, sg_sb[:, gcol])
                                nc.vector.tensor_add(dst[h][:, ocol], m1, sh)

                        # V for both heads: psum [s=128, 2*HD] accum over d_in
                        for s2 in range(4):
                            pv = p1psumv.tile([128, HPC * HD], F32, tag="v")
                            for i in range(16):
                                nc.tensor.matmul(
                                    pv,
                                    xt_sb[:, i, s2 * 128:(s2 + 1) * 128],
                                    wv_sb[:, i, :],
                                    start=(i == 0),
                                    stop=(i == 15),
                                )
                            sc = hf * 8 + j2 * 4 + s2
                            ccol = slice((b * NK + sc) * 128, (b * NK + sc + 1) * 128)
                            for h in range(HPC):
                                nc.scalar.copy(
                                    out=v_sb[h][:, ccol], in_=pv[:, h * HD:(h + 1) * HD]
                                )

            # ---------------- phases 2+3 SBUF pool ----------------
            with (
                tc.tile_pool(name="p23", bufs=1) as p23,
                tc.tile_pool(name="ptile", bufs=6) as ptile,
            ):
                wo_sb = p23.tile([128, 16, D], BF16, tag="wo")
                nc.scalar.dma_start(out=wo_sb, in_=wot[:, :, :])

                # ---------------- phase 2: attention (batch-major) ----------------
                with tc.tile_pool(name="p2psum", bufs=2, space="PSUM") as p2psum:
                    for b in range(B):
                        for h in range(HPC):
                            for jq in range(NB):
                                po = p2psum.tile([128, 512], F32, tag="pv", bufs=2)
                                pden = p2psum.tile([128, 512], F32, tag="den", bufs=2)
                                nkb = 4 * jq + 4  # causal: k-blocks 0..4jq+3
                                qcol = slice(b * S + jq * 512, b * S + (jq + 1) * 512)
                                # q-columns 256-511 only (high diagonal pair)
                                qcol_hi = slice(b * S + jq * 512 + 256, b * S + (jq + 1) * 512)
                                d_prev = None
                                dacc = None  # folded sum of all full-width pairs
                                for e in range(2 * jq):  # full off-diag pairs
                                    w = 512
                                    p_sb = ptile.tile([128, 2 * w], BF16, tag="p", name="p_sb")
                                    # one 2-bank score tile per pair (each
                                    # matmul stays inside its own bank) and a
                                    # single fused 1024-wide exp: scalar is
                                    # the phase-2 bottleneck, so halving the
                                    # ACTIVATE count beats finer pipelining
                                    psc = p2psum.tile([128, 2 * w], F32, tag="sc",
                                                      name="psc", bufs=2)
                                    for u in range(2):
                                        ik = 2 * e + u
                                        nc.tensor.matmul(
                                            psc[:, u * w:(u + 1) * w],
                                            k_sb[h][:, b * S + ik * 128: b * S + (ik + 1) * 128],
                                            q_sb[h][:, qcol],
                                            start=True,
                                            stop=True,
                                        )
                                    nc.scalar.activation(
                                        out=p_sb,
                                        in_=psc,
                                        func=mybir.ActivationFunctionType.Exp,
                                        scale=INV_SQRT_HD,
                                    )
                                    for u in range(2):
                                        ik = 2 * e + u
                                        vcol = slice((b * NK + ik) * 128, (b * NK + ik + 1) * 128)
                                        nc.tensor.matmul(
                                            po,
                                            v_sb[h][:, vcol], p_sb[:, u * w:(u + 1) * w],
                                            start=(ik == 0), stop=False,
                                            skip_group_check=True,
                                        )
                                    d_sb = tmp.tile([128, 512], BF16, tag="dpair")
                                    nc.vector.tensor_add(
                                        d_sb, p_sb[:, 0:w], p_sb[:, w:2 * w]
                                    )
                                    if e % 2 == 0:
                                        d_prev = d_sb  # defer: pair up with next
                                        continue
                                    dd = tmp.tile([128, 512], BF16, tag="dquad")
                                    nc.vector.tensor_add(dd, d_prev, d_sb)
                                    # fold everything full-width into one tile
                                    # on DVE so the denominator costs the
                                    # tensor engine just 512+256 cols per jq
                                    if dacc is None:
                                        dacc = dd
                                    else:
                                        f = tmp.tile([128, 512], BF16, tag="dquad")
                                        nc.vector.tensor_add(f, dacc, dd)
                                        dacc = f
                                # diagonal super-block: 4 k-blocks with
                                # per-block q-windows [128t, 512); only each
                                # window's leading 128 cols need the triangle
                                # mask, and score/PV columns shrink 1536->1280
                                DOFF = (0, 512, 896, 1152)
                                p_diag = ptile.tile([128, 1280], BF16, tag="pd", name="p_diag")
                                for t in range(4):
                                    ik = 4 * jq + t
                                    wt = 512 - 128 * t
                                    psc = p2psum.tile([128, wt], F32, tag="sc",
                                                      name="psc", bufs=2)
                                    nc.tensor.matmul(
                                        psc,
                                        k_sb[h][:, b * S + ik * 128: b * S + (ik + 1) * 128],
                                        q_sb[h][:, b * S + jq * 512 + 128 * t: b * S + (jq + 1) * 512],
                                        start=True,
                                        stop=True,
                                    )
                                    pd = p_diag[:, DOFF[t]:DOFF[t] + wt]
                                    nc.scalar.activation(
                                        out=pd,
                                        in_=psc,
                                        func=mybir.ActivationFunctionType.Exp,
                                        scale=INV_SQRT_HD,
                                    )
                                    nc.vector.tensor_mul(
                                        p_diag[:, DOFF[t]:DOFF[t] + 128],
                                        p_diag[:, DOFF[t]:DOFF[t] + 128],
                                        mask_sb[:, 0, 0:128],
                                    )
                                    nc.tensor.matmul(
                                        po[:, 128 * t:512],
                                        v_sb[h][:, (b * NK + ik) * 128:(b * NK + ik + 1) * 128],
                                        pd,
                                        start=(ik == 0), stop=(t == 3),
                                        skip_group_check=True,
                                    )
                                # denominator pieces: d01 covers the full 512
                                # window; d23 covers q-cols 256-511 (offset 0)
                                d01 = tmp.tile([128, 512], BF16, tag="dpair")
                                nc.vector.tensor_copy(
                                    out=d01[:, 0:128], in_=p_diag[:, 0:128]
                                )
                                nc.vector.tensor_add(
                                    d01[:, 128:512], p_diag[:, 128:512],
                                    p_diag[:, 512:896],
                                )
                                d23 = tmp.tile([128, 512], BF16, tag="dquad")
                                nc.vector.tensor_copy(
                                    out=d23[:, 0:128], in_=p_diag[:, 896:1024]
                                )
                                nc.vector.tensor_add(
                                    d23[:, 128:256], p_diag[:, 1024:1152],
                                    p_diag[:, 1152:1280],
                                )
                                if dacc is None:
                                    dacc = d01
                                else:
                                    f = tmp.tile([128, 512], BF16, tag="dquad")
                                    nc.vector.tensor_add(f, dacc, d01)
                                    dacc = f
                                nc.tensor.matmul(
                                    pden, ones_sb, dacc,
                                    start=True, stop=False,
                                    skip_group_check=True,
                                )
                                nc.tensor.matmul(
                                    pden[:, 256:512], ones_sb, d23[:, 0:256],
                                    start=False, stop=True,
                                    skip_group_check=True,
                                )
                                recip = tmp.tile([128, 512], F32, tag="recip")
                                nc.vector.reciprocal_approx_fast(out=recip, in_=pden)
                                ao = tmp.tile([128, 512], BF16, tag="ao")
                                nc.vector.tensor_mul(ao, po, recip)
                                for u in range(2):
                                    nc.gpsimd.dma_start(
                                        out=bnc_in[b][h][2 * jq + u, :, :],
                                        in_=ao[:, u * 256:(u + 1) * 256],
                                    )
                            # fire this (batch, head)'s AllToAll immediately;
                            # overlaps with the remaining attention compute
                            nc.gpsimd.collective_compute(
                                "AllToAll",
                                mybir.AluOpType.bypass,
                                replica_groups=[CORE_IDS],
                                ins=[bnc_in[b][h][:, :, :]],
                                outs=[bnc_out[b][h][:, :, :]],
                            )

                # ---------------- phase 3: output projection ----------------
                # gather DMAs on the (idle) sync queue, as soon as each
                # AllToAll lands; g{b}{h} holds heads of parity h for batch b.
                g_sb = [
                    [p23.tile([128, N_CORES, SCHUNK], BF16, tag=f"g{b}{h}",
                              name=f"g_sb{b}{h}") for h in range(HPC)]
                    for b in range(B)
                ]
                for b in range(B):
                    for h in range(HPC):
                        nc.sync.dma_start(
                            out=g_sb[b][h],
                            in_=bnc_out[b][h].rearrange("j p n -> p j n", p=128),
                        )
                # two passes over head parity: pass 0 uses only the h=0
                # AllToAll results (which land earlier), so ~half the output
                # projection can run before the last AllToAll completes.
                # NOTE: matmul start=True resets the whole PSUM *bank*, so
                # every concurrently-open accumulation group needs its own
                # bank: 8 groups of [128,256] at a time, two m-groups.
                with tc.tile_pool(name="p3psum", bufs=1, space="PSUM") as p3psum:
                    for b in range(B):
                        for mg in range(2):
                            pws = [
                                p3psum.tile([128, SCHUNK], F32, tag=f"wo{j}",
                                            name=f"pw{b}_{mg}_{j}")
                                for j in range(8)
                            ]
                            for par in range(2):
                                for j in range(8):
                                    m = mg * 8 + j
                                    pw = pws[j]
                                    for i2 in range(8):
                                        nc.tensor.matmul(
                                            pw,
                                            wo_sb[:, 2 * i2 + par, m * 128:(m + 1) * 128],
                                            g_sb[b][par][:, i2, :],
                                            start=(par == 0 and i2 == 0),
                                            stop=(par == 1 and i2 == 7),
                                            skip_group_check=True,
                                        )
                                    if par == 1:
                                        o_sb = tmp.tile([128, SCHUNK], BF16, tag="o")
                                        nc.vector.tensor_copy(out=o_sb, in_=pw)
                                        nc.scalar.dma_start(
                                            out=out_ext[m * 128:(m + 1) * 128, b * SCHUNK:(b + 1) * SCHUNK],
                                            in_=o_sb,
                                        )

    nc.compile()
    return nc


def prep_inputs(x, freqs_cos, freqs_sin, wq, wk, wv, wo):
    """Host-side shard prep. Returns in_maps (list of 8 dicts)."""
    bf = ml_dtypes.bfloat16
    x = np.asarray(x, dtype=np.float32)
    xtf = x.reshape(BS, D).T.astype(bf)  # [D, BS]
    # partition-major pre-chunk: [half, j2, p, i, n] -> fully sequential DMAs
    xt = np.ascontiguousarray(
        xtf.reshape(16, 128, 2 * B, 2, 512).transpose(2, 3, 1, 0, 4)
    )
    wot = np.ascontiguousarray(np.asarray(wo, np.float32).T.astype(bf).reshape(16, 128, D).transpose(1, 0, 2))
    cos = np.asarray(freqs_cos, np.float32)
    sin = np.asarray(freqs_sin, np.float32)
    cg = np.empty((HD, S), np.float32)
    sg = np.empty((HD, S), np.float32)
    cg[0::2] = cos.T
    cg[1::2] = cos.T
    sg[0::2] = -sin.T
    sg[1::2] = sin.T
    mk4 = np.zeros((4, HD, 512), np.float32)
    for t in range(4):
        kp = np.arange(HD)[:, None]
        qf = np.arange(512)[None, :]
        mk4[t] = (128 * t + kp <= qf).astype(np.float32)
    # mk[0]: low diagonal pair (blocks t0,t1) over full 512 q-cols;
    # mk[1][:, :512]: high pair (t2,t3) restricted to q-cols 256-511
    mk = np.zeros((2, HD, 1024), np.float32)
    mk[0][:, 0:512] = mk4[0]
    mk[0][:, 512:1024] = mk4[1]
    mk[1][:, 0:256] = mk4[2][:, 256:512]
    mk[1][:, 256:512] = mk4[3][:, 256:512]
    mk = np.ascontiguousarray(mk.astype(bf).transpose(1, 0, 2))

    def wchunks(w, rows):
        # [D_out rows slice].T -> [2048, 256] -> per-head [h, 128, 16, 128]
        wt = np.asarray(w, np.float32)[rows, :].T.astype(bf)  # [D, 256]
        return np.ascontiguousarray(
            wt.reshape(16, 128, HPC, HD).transpose(2, 1, 0, 3)
        )

    in_maps = []
    for c in range(N_CORES):
        rows = slice(c * HPC * HD, (c + 1) * HPC * HD)
        in_maps.append({
            "xt": xt,
            "wqt": wchunks(wq, rows),
            "wkt": wchunks(wk, rows),
            "wvt": wchunks(wv, rows),
            "wot": wot,
            "cgrid": cg,
            "sgrid": sg,
            "masks": mk,
        })
    return in_maps


def assemble(results):
    out = np.empty((B, S, D), np.float32)
    for c in range(N_CORES):
        r = np.asarray(results[c]["out"], dtype=np.float32)  # [D, B*SCHUNK]
        for b in range(B):
            out[b, c * SCHUNK:(c + 1) * SCHUNK, :] = (
                r[:, b * SCHUNK:(b + 1) * SCHUNK].T
            )
    return out


_NC_CACHE = []


def kernel(**inputs):
    """Full-input distributed attention on 8 TRN2 NeuronCores.

    Takes the unsharded inputs (x, freqs_cos, freqs_sin, wq, wk, wv, wo) as
    numpy float32 arrays, runs the SPMD bass kernel on cores 0-7, and
    returns the full [B, S, D] float32 output.
    """
    from concourse.bass_utils import run_bass_kernel_spmd

    if not _NC_CACHE:
        _NC_CACHE.append(build())
    nc = _NC_CACHE[0]
    in_maps = prep_inputs(
        x=inputs["x"],
        freqs_cos=inputs["freqs_cos"],
        freqs_sin=inputs["freqs_sin"],
        wq=inputs["wq"],
        wk=inputs["wk"],
        wv=inputs["wv"],
        wo=inputs["wo"],
    )
    try:
        res = run_bass_kernel_spmd(nc, in_maps, CORE_IDS, trace=False)
    except Exception:
        # transient NRT device hiccups recover on retry
        res = run_bass_kernel_spmd(nc, in_maps, CORE_IDS, trace=False)
    return assemble(res.results)


# revision 34
# speedup vs baseline: 1.0351x; 1.0351x over previous
"""Distributed multi-head causal attention for TRN2, 8 NeuronCores.

Strategy (tensor-parallel over heads + AllToAll re-shard for the output
projection):
  - Each core owns 2 of the 16 heads. It computes Q,K,V projections for its
    heads over the full sequence (both batches), applies RoPE, and computes
    causal softmax(QK^T/sqrt(hd)) @ V for its heads.
  - Everything on-chip is laid out TRANSPOSED: qT/kT are [hd, B*S], scores are
    [k, q], attention output is [hd, q].  This avoids all transposes:
      scoresT = kT_block.T @ qT        (lhsT=kT block, rhs=qT)
      aoT     = v_block.T  @ pT        (lhsT=v natural [k,hd], rhs=pT [k,q])
    Softmax denominator (sum over k = partition axis) comes from a ones-matmul
    (lhsT=ones [128,128]) that also broadcasts the sum across partitions.
    exp() is computed WITHOUT max subtraction (max |score| ~ 6, safe in f32).
  - Four AllToAlls (one per (batch, head)) swap head-sharding for
    sequence-sharding; each fires as soon as that (batch, head)'s attention
    output is stored, overlapping the collective with the remaining attention
    compute.  Each core ends with all 16 heads for its 256-position slice of
    each batch, then computes its slice of the wo projection:
    outT = woT_chunk.T @ attnT (output transposed; host transposes back).
    The wo contraction runs in two passes over head parity (one PSUM bank
    per open accumulation group — start=True resets the whole bank), so the
    even-head half can run before the last AllToAll lands.
  - Compute dtype: bf16 matmul operands, f32 PSUM accumulation, f32 softmax,
    bf16 output (host upcasts; well within tolerance).
  - Startup: sync+scalar DMA queues carry only the x stream, gpsimd carries
    weights/grids/mask in demand order, so the first QK accumulation starts
    as early as the DMA-latency floor allows.

Host-side prep casts inputs to bf16 and pre-transposes x/wo; host-side
assembly transposes/concats per-core outputs.  No host arithmetic.
"""
import math

import ml_dtypes
import numpy as np

import concourse.bass as bass
import concourse.mybir as mybir
from concourse import bacc
from concourse.tile import TileContext

F32 = mybir.dt.float32
BF16 = mybir.dt.bfloat16

N_CORES = 8
CORE_IDS = list(range(N_CORES))
B = 2
S = 2048
D = 2048
H = 16
HD = 128  # head dim
HPC = H // N_CORES  # heads per core = 2
BS = B * S  # 4096
NB = S // 512  # 4 q-free-blocks per batch
NK = S // 128  # 16 k-blocks per batch
SCHUNK = S // N_CORES  # 256 positions per core per batch
INV_SQRT_HD = 1.0 / math.sqrt(HD)

# stream_shuffle mask: swap adjacent partitions within each 32-group
PAIR_SWAP = [i ^ 1 for i in range(32)]


def build():
    nc = bacc.Bacc(None, num_devices=N_CORES)

    # x pre-transposed/chunked: [half, j2, p, i, n] fully contiguous per
    # (half, j2) so startup DMAs are large sequential reads.
    xt = nc.declare_dram_parameter("xt", [2 * B, 2, 128, 16, 512], BF16, isOutput=False)
    # per-head weight chunks: [h, p, i, hd] contiguous per head.
    wqt = nc.declare_dram_parameter("wqt", [HPC, 128, 16, HD], BF16, isOutput=False)
    wkt = nc.declare_dram_parameter("wkt", [HPC, 128, 16, HD], BF16, isOutput=False)
    wvt = nc.declare_dram_parameter("wvt", [HPC, 128, 16, HD], BF16, isOutput=False)
    wot = nc.declare_dram_parameter("wot", [128, 16, D], BF16, isOutput=False)
    cgrid = nc.declare_dram_parameter("cgrid", [HD, S], F32, isOutput=False)
    sgrid = nc.declare_dram_parameter("sgrid", [HD, S], F32, isOutput=False)
    masks = nc.declare_dram_parameter("masks", [HD, 2, 1024], BF16, isOutput=False)
    out_ext = nc.declare_dram_parameter("out", [D, B * SCHUNK], BF16, isOutput=True)

    bnc_in = [
        [nc.dram_tensor(f"bounce_in{b}_{h}", [N_CORES, HD, SCHUNK], BF16)
         for h in range(HPC)]
        for b in range(B)
    ]
    bnc_out = [
        [nc.dram_tensor(f"bounce_out{b}_{h}", [N_CORES, HD, SCHUNK], BF16)
         for h in range(HPC)]
        for b in range(B)
    ]

    bar_in = nc.dram_tensor("bar_in", [1], F32)
    bar_out = nc.dram_tensor("bar_out", [N_CORES], F32, addr_space="Shared")

    with TileContext(nc) as tc:
        with (
            tc.tile_pool(name="persist", bufs=1) as persist,
            tc.tile_pool(name="tmp", bufs=4) as tmp,
        ):
            # ---------------- persistent SBUF tensors ----------------
            mask_sb = persist.tile([128, 2, 1024], BF16, tag="mask")
            ones_sb = persist.tile([128, 128], BF16, tag="ones")
            nc.vector.memset(ones_sb, 1.0)

            # qT/kT per head: [hd=128, BS] bf16 (post-RoPE).
            # v per head: [128, BS] bf16, chunk ik at cols [128*ik,128*(ik+1))
            # holding v rows (k) on partitions, hd on free.
            q_sb = [persist.tile([128, BS], BF16, tag=f"q{h}", name=f"q_sb{h}") for h in range(HPC)]
            k_sb = [persist.tile([128, BS], BF16, tag=f"k{h}", name=f"k_sb{h}") for h in range(HPC)]
            v_sb = [persist.tile([128, BS], BF16, tag=f"v{h}", name=f"v_sb{h}") for h in range(HPC)]

            # ---------------- phase 1: QKV projections + RoPE ----------------
            with (
                tc.tile_pool(name="p1w", bufs=1) as p1w,
                tc.tile_pool(name="xt_pool", bufs=2) as xt_pool,
                tc.tile_pool(name="p1psum", bufs=1, space="PSUM") as p1psum,
                tc.tile_pool(name="p1psumv", bufs=2, space="PSUM") as p1psumv,
            ):
                wq_sb = p1w.tile([128, 16, HPC * HD], BF16, tag="wq")
                wk_sb = p1w.tile([128, 16, HPC * HD], BF16, tag="wk")
                wv_sb = p1w.tile([128, 16, HPC * HD], BF16, tag="wv")
                cg_sb = p1w.tile([128, S], F32, tag="cg")
                sg_sb = p1w.tile([128, S], F32, tag="sg")

                # Startup DMA priority: the sync+scalar queues carry ONLY the
                # xt stream (so nothing backs it up); gpsimd carries weights +
                # grids + mask in exact demand order.  The first xt chunk is
                # split 4 ways i-ascending so the first QK accumulation can
                # begin as soon as the first i-chunks land.
                # Startup DMA priority: sync+scalar carry ONLY the xt stream;
                # gpsimd carries weights + grids + mask in demand order.
                xt00_sb = xt_pool.tile([128, 16, 512], BF16, tag="xt")
                nc.sync.dma_start(out=xt00_sb[:, 0:4, :], in_=xt[0, 0, :, 0:4, :])
                nc.scalar.dma_start(out=xt00_sb[:, 4:8, :], in_=xt[0, 0, :, 4:8, :])
                nc.sync.dma_start(out=xt00_sb[:, 8:12, :], in_=xt[0, 0, :, 8:12, :])
                nc.scalar.dma_start(out=xt00_sb[:, 12:16, :], in_=xt[0, 0, :, 12:16, :])
                nc.gpsimd.dma_start(out=wq_sb[:, :, 0:HD], in_=wqt[0])
                # dummy AllGather: absorbs cross-core NEFF-launch skew early,
                # so the later AllToAlls see aligned peers
                nc.gpsimd.collective_compute(
                    "AllGather",
                    mybir.AluOpType.bypass,
                    replica_groups=[CORE_IDS],
                    ins=[bar_in[:]],
                    outs=[bar_out[:]],
                )
                nc.gpsimd.dma_start(out=wq_sb[:, :, HD:2 * HD], in_=wqt[1])
                nc.gpsimd.dma_start(out=wk_sb[:, :, 0:HD], in_=wkt[0])
                nc.gpsimd.dma_start(out=wk_sb[:, :, HD:2 * HD], in_=wkt[1])
                # RoPE grids in demand order: first 512 cols feed the
                # first RoPE block; the rest can trickle in later.
                nc.gpsimd.dma_start(out=cg_sb[:, 0:512], in_=cgrid[:, 0:512])
                nc.gpsimd.dma_start(out=sg_sb[:, 0:512], in_=sgrid[:, 0:512])
                nc.gpsimd.dma_start(out=wv_sb[:, :, 0:HD], in_=wvt[0])
                nc.gpsimd.dma_start(out=wv_sb[:, :, HD:2 * HD], in_=wvt[1])
                nc.gpsimd.dma_start(out=cg_sb[:, 512:1024], in_=cgrid[:, 512:1024])
                nc.gpsimd.dma_start(out=sg_sb[:, 512:1024], in_=sgrid[:, 512:1024])
                nc.gpsimd.dma_start(out=cg_sb[:, 1024:2048], in_=cgrid[:, 1024:2048])
                nc.gpsimd.dma_start(out=sg_sb[:, 1024:2048], in_=sgrid[:, 1024:2048])
                nc.gpsimd.dma_start(out=mask_sb, in_=masks[:, :, :])

                # HAM clock-gate warmup: the PE runs at 1.2 GHz until it sees
                # ~3.4us of sustained activity, and re-cools after ~3.4us
                # idle.  The first real matmul waits ~14us for DMA; fill that
                # whole shadow with dummy matmuls on DMA-free tiles so real
                # work starts at full clock (cold: ~12 mm @427ns, then warm
                # @213ns up to ~22.5us).
                warm_src = p1w.tile([128, 512], BF16, tag="warmsrc")
                nc.vector.memset(warm_src, 1.0)
                warm = p1psum.tile([128, 512], F32, tag="warm")
                for i in range(68):
                    nc.tensor.matmul(
                        warm, ones_sb, warm_src,
                        start=(i == 0), stop=(i == 67),
                    )

                for half in range(2 * B):  # half-batches of 1024 positions
                    b, hf = divmod(half, 2)
                    for j2 in range(2):
                        coff = b * S + hf * 1024 + j2 * 512  # col off in [D, BS]
                        poff = hf * 1024 + j2 * 512  # position offset (grids)
                        if half == 0 and j2 == 0:
                            xt_sb = xt00_sb
                        else:
                            xt_sb = xt_pool.tile([128, 16, 512], BF16, tag="xt")
                            nc.sync.dma_start(
                                out=xt_sb[:, 0:8, :], in_=xt[half, j2, :, 0:8, :]
                            )
                            nc.scalar.dma_start(
                                out=xt_sb[:, 8:16, :], in_=xt[half, j2, :, 8:16, :]
                            )

                        # Q, K for both heads: psum [hd, 512] accum over d_in
                        ps = {}
                        for kind, w in (("q", wq_sb), ("k", wk_sb)):
                            for h in range(HPC):
                                p = p1psum.tile([128, 512], F32, tag=f"qk{kind}{h}")
                                ps[(kind, h)] = p
                                for i in range(16):
                                    nc.tensor.matmul(
                                        p,
                                        w[:, i, h * HD:(h + 1) * HD],
                                        xt_sb[:, i, :],
                                        start=(i == 0),
                                        stop=(i == 15),
                                    )
                        # RoPE: out = t*cos + pairswap(t)*sin_signed (DVE only)
                        gcol = slice(poff, poff + 512)
                        ocol = slice(coff, coff + 512)
                        for kind, dst in (("q", q_sb), ("k", k_sb)):
                            for h in range(HPC):
                                p = ps[(kind, h)]
                                m1 = tmp.tile([128, 512], F32, tag="rope_m1")
                                nc.vector.tensor_mul(m1, p, cg_sb[:, gcol])
                                sh = tmp.tile([128, 512], F32, tag="rope_sh")
                                nc.vector.stream_shuffle(sh, p, PAIR_SWAP)
                                nc.vector.tensor_mul(sh, sh, sg_sb[:, gcol])
                                nc.vector.tensor_add(dst[h][:, ocol], m1, sh)

                        # V for both heads: psum [s=128, 2*HD] accum over d_in
                        for s2 in range(4):
                            pv = p1psumv.tile([128, HPC * HD], F32, tag="v")
                            for i in range(16):
                                nc.tensor.matmul(
                                    pv,
                                    xt_sb[:, i, s2 * 128:(s2 + 1) * 128],
                                    wv_sb[:, i, :],
                                    start=(i == 0),
                                    stop=(i == 15),
                                )
                            sc = hf * 8 + j2 * 4 + s2
                            ccol = slice((b * NK + sc) * 128, (b * NK + sc + 1) * 128)
                            for h in range(HPC):
                                nc.scalar.copy(
                                    out=v_sb[h][:, ccol], in_=pv[:, h * HD:(h + 1) * HD]
                                )

            # ---------------- phases 2+3 SBUF pool ----------------
            with (
                tc.tile_pool(name="p23", bufs=1) as p23,
                tc.tile_pool(name="ptile", bufs=6) as ptile,
            ):
                wo_sb = p23.tile([128, 16, D], BF16, tag="wo")
                nc.scalar.dma_start(out=wo_sb, in_=wot[:, :, :])

                # ---------------- phase 2: attention (batch-major) ----------------
                with tc.tile_pool(name="p2psum", bufs=2, space="PSUM") as p2psum:
                    for b in range(B):
                        for h in range(HPC):
                            for jq in range(NB):
                                po = p2psum.tile([128, 512], F32, tag="pv", bufs=2)
                                pden = p2psum.tile([128, 512], F32, tag="den", bufs=2)
                                nkb = 4 * jq + 4  # causal: k-blocks 0..4jq+3
                                qcol = slice(b * S + jq * 512, b * S + (jq + 1) * 512)
                                # q-columns 256-511 only (high diagonal pair)
                                qcol_hi = slice(b * S + jq * 512 + 256, b * S + (jq + 1) * 512)
                                d_prev = None
                                dacc = None  # folded sum of all full-width pairs
                                for e in range(2 * jq):  # full off-diag pairs
                                    w = 512
                                    p_sb = ptile.tile([128, 2 * w], BF16, tag="p", name="p_sb")
                                    for u in range(2):
                                        ik = 2 * e + u
                                        psc = p2psum.tile([128, w], F32, tag="sc",
                                                          name="psc", bufs=4)
                                        nc.tensor.matmul(
                                            psc,
                                            k_sb[h][:, b * S + ik * 128: b * S + (ik + 1) * 128],
                                            q_sb[h][:, qcol],
                                            start=True,
                                            stop=True,
                                        )
                                        nc.scalar.activation(
                                            out=p_sb[:, u * w:(u + 1) * w],
                                            in_=psc,
                                            func=mybir.ActivationFunctionType.Exp,
                                            scale=INV_SQRT_HD,
                                        )
                                    for u in range(2):
                                        ik = 2 * e + u
                                        vcol = slice((b * NK + ik) * 128, (b * NK + ik + 1) * 128)
                                        nc.tensor.matmul(
                                            po,
                                            v_sb[h][:, vcol], p_sb[:, u * w:(u + 1) * w],
                                            start=(ik == 0), stop=False,
                                            skip_group_check=True,
                                        )
                                    d_sb = tmp.tile([128, 512], BF16, tag="dpair")
                                    nc.vector.tensor_add(
                                        d_sb, p_sb[:, 0:w], p_sb[:, w:2 * w]
                                    )
                                    if e % 2 == 0:
                                        d_prev = d_sb  # defer: pair up with next
                                        continue
                                    dd = tmp.tile([128, 512], BF16, tag="dquad")
                                    nc.vector.tensor_add(dd, d_prev, d_sb)
                                    # fold everything full-width into one tile
                                    # on DVE so the denominator costs the
                                    # tensor engine just 512+256 cols per jq
                                    if dacc is None:
                                        dacc = dd
                                    else:
                                        f = tmp.tile([128, 512], BF16, tag="dquad")
                                        nc.vector.tensor_add(f, dacc, dd)
                                        dacc = f
                                # diagonal super-block: 4 k-blocks with
                                # per-block q-windows [128t, 512); only each
                                # window's leading 128 cols need the triangle
                                # mask, and score/PV columns shrink 1536->1280
                                DOFF = (0, 512, 896, 1152)
                                p_diag = ptile.tile([128, 1280], BF16, tag="pd", name="p_diag")
                                for t in range(4):
                                    ik = 4 * jq + t
                                    wt = 512 - 128 * t
                                    psc = p2psum.tile([128, wt], F32, tag="sc",
                                                      name="psc", bufs=4)
                                    nc.tensor.matmul(
                                        psc,
                                        k_sb[h][:, b * S + ik * 128: b * S + (ik + 1) * 128],
                                        q_sb[h][:, b * S + jq * 512 + 128 * t: b * S + (jq + 1) * 512],
                                        start=True,
                                        stop=True,
                                    )
                                    pd = p_diag[:, DOFF[t]:DOFF[t] + wt]
                                    nc.scalar.activation(
                                        out=pd,
                                        in_=psc,
                                        func=mybir.ActivationFunctionType.Exp,
                                        scale=INV_SQRT_HD,
                                    )
                                    nc.vector.tensor_mul(
                                        p_diag[:, DOFF[t]:DOFF[t] + 128],
                                        p_diag[:, DOFF[t]:DOFF[t] + 128],
                                        mask_sb[:, 0, 0:128],
                                    )
                                    nc.tensor.matmul(
                                        po[:, 128 * t:512],
                                        v_sb[h][:, (b * NK + ik) * 128:(b * NK + ik + 1) * 128],
                                        pd,
                                        start=(ik == 0), stop=(t == 3),
                                        skip_group_check=True,
                                    )
                                # denominator pieces: d01 covers the full 512
                                # window; d23 covers q-cols 256-511 (offset 0)
                                d01 = tmp.tile([128, 512], BF16, tag="dpair")
                                nc.vector.tensor_copy(
                                    out=d01[:, 0:128], in_=p_diag[:, 0:128]
                                )
                                nc.vector.tensor_add(
                                    d01[:, 128:512], p_diag[:, 128:512],
                                    p_diag[:, 512:896],
                                )
                                d23 = tmp.tile([128, 512], BF16, tag="dquad")
                                nc.vector.tensor_copy(
                                    out=d23[:, 0:128], in_=p_diag[:, 896:1024]
                                )
                                nc.vector.tensor_add(
                                    d23[:, 128:256], p_diag[:, 1024:1152],
                                    p_diag[:, 1152:1280],
                                )
                                if dacc is None:
                                    dacc = d01
                                else:
                                    f = tmp.tile([128, 512], BF16, tag="dquad")
                                    nc.vector.tensor_add(f, dacc, d01)
                                    dacc = f
                                nc.tensor.matmul(
                                    pden, ones_sb, dacc,
                                    start=True, stop=False,
                                    skip_group_check=True,
                                )
                                nc.tensor.matmul(
                                    pden[:, 256:512], ones_sb, d23[:, 0:256],
                                    start=False, stop=True,
                                    skip_group_check=True,
                                )
                                recip = tmp.tile([128, 512], F32, tag="recip")
                                nc.vector.reciprocal_approx_fast(out=recip, in_=pden)
                                ao = tmp.tile([128, 512], BF16, tag="ao")
                                nc.vector.tensor_mul(ao, po, recip)
                                for u in range(2):
                                    nc.gpsimd.dma_start(
                                        out=bnc_in[b][h][2 * jq + u, :, :],
                                        in_=ao[:, u * 256:(u + 1) * 256],
                                    )
                            # fire this (batch, head)'s AllToAll immediately;
                            # overlaps with the remaining attention compute
                            nc.gpsimd.collective_compute(
                                "AllToAll",
                                mybir.AluOpType.bypass,
                                replica_groups=[CORE_IDS],
                                ins=[bnc_in[b][h][:, :, :]],
                                outs=[bnc_out[b][h][:, :, :]],
                            )

                # ---------------- phase 3: output projection ----------------
                # gather DMAs on the (idle) sync queue, as soon as each
                # AllToAll lands; g{b}{h} holds heads of parity h for batch b.
                g_sb = [
                    [p23.tile([128, N_CORES, SCHUNK], BF16, tag=f"g{b}{h}",
                              name=f"g_sb{b}{h}") for h in range(HPC)]
                    for b in range(B)
                ]
                for b in range(B):
                    for h in range(HPC):
                        nc.sync.dma_start(
                            out=g_sb[b][h],
                            in_=bnc_out[b][h].rearrange("j p n -> p j n", p=128),
                        )
                # two passes over head parity: pass 0 uses only the h=0
                # AllToAll results (which land earlier), so ~half the output
                # projection can run before the last AllToAll completes.
                # NOTE: matmul start=True resets the whole PSUM *bank*, so
                # every concurrently-open accumulation group needs its own
                # bank: 8 groups of [128,256] at a time, two m-groups.
                with tc.tile_pool(name="p3psum", bufs=1, space="PSUM") as p3psum:
                    for b in range(B):
                        for mg in range(2):
                            pws = [
                                p3psum.tile([128, SCHUNK], F32, tag=f"wo{j}",
                                            name=f"pw{b}_{mg}_{j}")
                                for j in range(8)
                            ]
                            for par in range(2):
                                for j in range(8):
                                    m = mg * 8 + j
                                    pw = pws[j]
                                    for i2 in range(8):
                                        nc.tensor.matmul(
                                            pw,
                                            wo_sb[:, 2 * i2 + par, m * 128:(m + 1) * 128],
                                            g_sb[b][par][:, i2, :],
                                            start=(par == 0 and i2 == 0),
                                            stop=(par == 1 and i2 == 7),
                                            skip_group_check=True,
                                        )
                                    if par == 1:
                                        o_sb = tmp.tile([128, SCHUNK], BF16, tag="o")
                                        nc.vector.tensor_copy(out=o_sb, in_=pw)
                                        nc.scalar.dma_start(
                                            out=out_ext[m * 128:(m + 1) * 128, b * SCHUNK:(b + 1) * SCHUNK],
                                            in_=o_sb,
                                        )

    nc.compile()
    return nc


def prep_inputs(x, freqs_cos, freqs_sin, wq, wk, wv, wo):
    """Host-side shard prep. Returns in_maps (list of 8 dicts)."""
    bf = ml_dtypes.bfloat16
    x = np.asarray(x, dtype=np.float32)
    xtf = x.reshape(BS, D).T.astype(bf)  # [D, BS]
    # partition-major pre-chunk: [half, j2, p, i, n] -> fully sequential DMAs
    xt = np.ascontiguousarray(
        xtf.reshape(16, 128, 2 * B, 2, 512).transpose(2, 3, 1, 0, 4)
    )
    wot = np.ascontiguousarray(np.asarray(wo, np.float32).T.astype(bf).reshape(16, 128, D).transpose(1, 0, 2))
    cos = np.asarray(freqs_cos, np.float32)
    sin = np.asarray(freqs_sin, np.float32)
    cg = np.empty((HD, S), np.float32)
    sg = np.empty((HD, S), np.float32)
    cg[0::2] = cos.T
    cg[1::2] = cos.T
    sg[0::2] = -sin.T
    sg[1::2] = sin.T
    mk4 = np.zeros((4, HD, 512), np.float32)
    for t in range(4):
        kp = np.arange(HD)[:, None]
        qf = np.arange(512)[None, :]
        mk4[t] = (128 * t + kp <= qf).astype(np.float32)
    # mk[0]: low diagonal pair (blocks t0,t1) over full 512 q-cols;
    # mk[1][:, :512]: high pair (t2,t3) restricted to q-cols 256-511
    mk = np.zeros((2, HD, 1024), np.float32)
    mk[0][:, 0:512] = mk4[0]
    mk[0][:, 512:1024] = mk4[1]
    mk[1][:, 0:256] = mk4[2][:, 256:512]
    mk[1][:, 256:512] = mk4[3][:, 256:512]
    mk = np.ascontiguousarray(mk.astype(bf).transpose(1, 0, 2))

    def wchunks(w, rows):
        # [D_out rows slice].T -> [2048, 256] -> per-head [h, 128, 16, 128]
        wt = np.asarray(w, np.float32)[rows, :].T.astype(bf)  # [D, 256]
        return np.ascontiguousarray(
            wt.reshape(16, 128, HPC, HD).transpose(2, 1, 0, 3)
        )

    in_maps = []
    for c in range(N_CORES):
        rows = slice(c * HPC * HD, (c + 1) * HPC * HD)
        in_maps.append({
            "xt": xt,
            "wqt": wchunks(wq, rows),
            "wkt": wchunks(wk, rows),
            "wvt": wchunks(wv, rows),
            "wot": wot,
            "cgrid": cg,
            "sgrid": sg,
            "masks": mk,
        })
    return in_maps


def assemble(results):
    out = np.empty((B, S, D), np.float32)
    for c in range(N_CORES):
        r = np.asarray(results[c]["out"], dtype=np.float32)  # [D, B*SCHUNK]
        for b in range(B):
            out[b, c * SCHUNK:(c + 1) * SCHUNK, :] = (
                r[:, b * SCHUNK:(b + 1) * SCHUNK].T
            )
    return out


_NC_CACHE = []


def kernel(**inputs):
    """Full-input distributed attention on 8 TRN2 NeuronCores.

    Takes the unsharded inputs (x, freqs_cos, freqs_sin, wq, wk, wv, wo) as
    numpy float32 arrays, runs the SPMD bass kernel on cores 0-7, and
    returns the full [B, S, D] float32 output.
    """
    from concourse.bass_utils import run_bass_kernel_spmd

    if not _NC_CACHE:
        _NC_CACHE.append(build())
    nc = _NC_CACHE[0]
    in_maps = prep_inputs(
        x=inputs["x"],
        freqs_cos=inputs["freqs_cos"],
        freqs_sin=inputs["freqs_sin"],
        wq=inputs["wq"],
        wk=inputs["wk"],
        wv=inputs["wv"],
        wo=inputs["wo"],
    )
    try:
        res = run_bass_kernel_spmd(nc, in_maps, CORE_IDS, trace=False)
    except Exception:
        # transient NRT device hiccups recover on retry
        res = run_bass_kernel_spmd(nc, in_maps, CORE_IDS, trace=False)
    return assemble(res.results)
